# revision 1
# baseline (speedup 1.0000x reference)
"""CliffordLinear forward on 8 Trainium2 NeuronCores.

The reference computes, for x:[4096,512,8,8], weight:[8,8,8], bias:[8,8]:

    out[b, o, k] = sum_{i,q,p} T[k,p,q] * weight[o,i,p] * x[b, i, q] + bias[o,k]

which is a single GEMM over the flattened feature dims:

    out_flat[b, (o,k)] = x_flat[b, (i,q)] @ M[(i,q), (o,k)] + bias_flat[(o,k)]
    M[(i,q), (o,k)]    = sum_p T[k,p,q] * weight[o,i,p]      (dense 64x64)

Strategy (data-parallel over the batch dim, per the sharding hint):
  - Host: build M (tiny), shard x_flat [2M, 64] into 8 equal row blocks.
  - Host: pack each shard into [TILES, 128, NT] "feature-major" tiles so the
    device sees the contraction dim (64) on SBUF partitions with fully
    contiguous 8KB-per-partition DMA lines: tile t holds two column blocks of
    xT = x_flat.T, one on partitions 0:64 and one on partitions 64:128.
  - Device (SPMD, identical NEFF on cores 0-7): stream tiles in via HWDGE DMA,
    one fp32 matmul per (half, 512-col chunk) with M stationary in the PE
    array (lhsT = M, K=64), PSUM -> SBUF copy fused with the bias add
    (alternating ScalarE / VectorE), stream out.
  - Host: unpack to [4096, 512, 8, 8].

The kernel is memory-bound: 64 MB in + 64 MB out per core at ~360-420 GB/s.
"""

import os

import numpy as np

import concourse.bass as bass
import concourse.mybir as mybir
import concourse.tile as tile
from concourse.bass_utils import run_bass_kernel_spmd

N_CORES = 8
B_TOTAL = 4096 * 512
B_CORE = B_TOTAL // N_CORES  # 262144 rows per core

# batch columns per partition half per DMA tile, by build variant
_VARIANT_NT = {"f32r_small": 1024, "f32r_big": 4096, "f32r_big8": 8192}


def _nt_for(variant: str) -> int:
    return _VARIANT_NT.get(variant, 2048)

# Cl(3,0) structure constants: (a x b)_k = sum_{p,q} T[k,p,q] a_p b_q.
_TERMS = [
    (0,0,0, 1),(0,1,1, 1),(0,2,2, 1),(0,3,3, 1),(0,4,4,-1),(0,5,5,-1),(0,6,6,-1),(0,7,7,-1),
    (1,0,1, 1),(1,1,0, 1),(1,2,4,-1),(1,3,5, 1),(1,4,2, 1),(1,5,3,-1),(1,6,7,-1),(1,7,6,-1),
    (2,0,2, 1),(2,1,4, 1),(2,2,0, 1),(2,3,6,-1),(2,4,1,-1),(2,5,7, 1),(2,6,3, 1),(2,7,5,-1),
    (3,0,3, 1),(3,1,5,-1),(3,2,6, 1),(3,3,0, 1),(3,4,7,-1),(3,5,1,-1),(3,6,2,-1),(3,7,4, 1),
    (4,0,4, 1),(4,1,2, 1),(4,2,1,-1),(4,3,7, 1),(4,4,0, 1),(4,5,6,-1),(4,6,5, 1),(4,7,3,-1),
    (5,0,5, 1),(5,1,3,-1),(5,2,7, 1),(5,3,1, 1),(5,4,6, 1),(5,5,0, 1),(5,6,4,-1),(5,7,2,-1),
    (6,0,6, 1),(6,1,7,-1),(6,2,3,-1),(6,3,2, 1),(6,4,5,-1),(6,5,4, 1),(6,6,0, 1),(6,7,1, 1),
    (7,0,7, 1),(7,1,6, 1),(7,2,5,-1),(7,3,4, 1),(7,4,3, 1),(7,5,2,-1),(7,6,1, 1),(7,7,0, 1),
]

# Results of the most recent run_bass_kernel_spmd call (for test harnesses
# that want exec_time_ns / trace paths).
LAST_RESULTS = None

_NC_CACHE = None

# Kernel build variant: "f32r_split" (single-pass float32r matmuls, psum at
# partitions 0:64, split copies) or "f32_classic" (2-pass fp32 matmuls, psum
# across all 128 partitions, one copy per tile on alternating engines).
VARIANT = os.environ.get("CLIFFORD_KERNEL_VARIANT", "f32r_split")


def _build_m(weight: np.ndarray) -> np.ndarray:
    t = np.zeros((8, 8, 8), np.float32)
    for k, p, q, s in _TERMS:
        t[k, p, q] = s
    m = np.einsum("kpq,oip->iqok", t, weight.astype(np.float32))
    return np.ascontiguousarray(m.reshape(64, 64), dtype=np.float32)


def _split_excess_waits(nc: bass.Bass, max_waits: int = 1) -> None:
    """Walrus limits the number of sync-wait commands per lowered instruction
    (1 for the PE LDWEIGHTS struct; the tile-context tail Drain with 9+ waits
    also overflows). Move excess waits onto preceding same-engine NOPs, which
    execute their waits in program order before the instruction."""
    pe_ops = ("Matmult", "Ldweights")
    n = 0
    for f in nc.m.functions:
        for blk in f.blocks:
            il = blk.instructions
            idx = 0
            while idx < len(il):
                inst = il[idx]
                si = inst.sync_info
                if si is None or not si.on_wait:
                    idx += 1
                    continue
                limit = 1 if inst.opcode in pe_ops else max_waits
                waits = list(si.on_wait)
                if len(waits) <= limit:
                    idx += 1
                    continue
                keep = waits[-limit:]
                extra = waits[:-limit]
                for j in range(0, len(extra), max_waits):
                    n += 1
                    nop = mybir.InstNoOp(
                        name=f"I-waitsplit-{n}",
                        sync_info=mybir.SyncInfo(
                            on_wait=extra[j : j + max_waits], on_update=[]
                        ),
                        bass_nofuse=True,
                        engine=inst.engine,
                    )
                    il.insert(idx, nop)
                    idx += 1
                inst.sync_info = mybir.SyncInfo(on_wait=keep, on_update=si.on_update)
                idx += 1


def _build_bass(variant: str | None = None) -> bass.Bass:
    variant = variant or VARIANT
    NT = _nt_for(variant)
    TILES = B_CORE // (2 * NT)
    nc = bass.Bass()
    dt = mybir.dt.float32
    # float32r (e8m11, single-PE-pass fp32) inputs: same 4-byte words, host
    # pre-rounds to 11 mantissa bits so the PE sees well-defined values.
    dtr = mybir.dt.float32r if variant.startswith("f32r") else dt
    xd = nc.dram_tensor("xd", [TILES, 128, NT], dtr, kind="ExternalInput")
    wd = nc.dram_tensor("wd", [128, 64], dtr, kind="ExternalInput")
    bd = nc.dram_tensor("bd", [128, 1], dt, kind="ExternalInput")
    od = nc.dram_tensor("od", [TILES, 128, NT], dt, kind="ExternalOutput")

    with tile.TileContext(nc) as tc:
        with (
            tc.tile_pool(name="cpool", bufs=1) as cpool,
            tc.tile_pool(name="iopool", bufs=6) as iopool,
            tc.tile_pool(name="pspool", bufs=2, space="PSUM") as pspool,
        ):
            w_sb = cpool.tile([128, 64], dtr)
            nc.sync.dma_start(w_sb, wd[:])
            b_sb = cpool.tile([128, 1], dt)
            nc.sync.dma_start(b_sb, bd[:])

            # Prologue touches fold the bias DMA wait into each copy
            # engine's clock once, instead of onto a steady-state op.
            # (_split_excess_waits legalizes any remaining multi-wait
            # instruction by spilling waits onto same-engine NOPs.)
            scr_a = cpool.tile([128, 1], dt)
            nc.scalar.copy(scr_a, b_sb)
            scr_v = cpool.tile([128, 1], dt)
            nc.vector.tensor_copy(scr_v, b_sb)

            io_bufs = {"f32r_big8": (3, 2)}.get(variant)
            for t in range(TILES):
                xt = iopool.tile(
                    [128, NT], dtr, bufs=io_bufs[0] if io_bufs else None
                )
                if variant == "f32r_sw2" and t % 2 == 1:
                    nc.scalar.dma_start(xt, xd[t])
                else:
                    nc.sync.dma_start(xt, xd[t])

                # float32r matmuls (single PE pass, 1 cycle/row at N>=512 vs
                # 4 for plain fp32) must write PSUM starting at partition 0,
                # so each half c gets its own [64, NT] psum tile; the
                # copy+bias step re-assembles the [128, NT] output tile,
                # shifting c=1 up to partitions 64:128.
                if variant.startswith("f32r"):
                    # PSUM budget caps each psum tile at 2048 f32 columns
                    # (4 banks) x2 halves x bufs=2 = 8 banks; larger DMA
                    # tiles are processed in 2048-column sub-tiles.
                    ns = min(NT, 2048)
                    ot = iopool.tile(
                        [128, NT], dt, bufs=io_bufs[1] if io_bufs else None
                    )
                    for s in range(NT // ns):
                        pss = [
                            pspool.tile(
                                [64, ns], dt, name=f"ps_{t}_{s}_{c}", tag="ps"
                            )
                            for c in (0, 1)
                        ]
                        for c in (0, 1):
                            lo = 64 * c
                            for j in range(ns // 512):
                                col = s * ns + 512 * j
                                nc.tensor.matmul(
                                    pss[c][0:64, 512 * j : 512 * (j + 1)],
                                    w_sb[lo : lo + 64, :],
                                    xt[lo : lo + 64, col : col + 512],
                                    start=True,
                                    stop=True,
                                )
                        # Copy+bias, re-assembling [128, ns]: ScalarE takes
                        # the lower half, VectorE the shifted upper half.
                        nc.scalar.add(
                            ot[0:64, s * ns : (s + 1) * ns],
                            pss[0][0:64, :],
                            add=b_sb[0:64, :],
                        )
                        nc.vector.tensor_scalar_add(
                            ot[64:128, s * ns : (s + 1) * ns],
                            pss[1][0:64, :],
                            b_sb[0:64, :],
                        )
                    if variant == "f32r_sw" and t % 2 == 1:
                        # Alternate stores onto the SWDGE path so two
                        # independent issuers drain the output stream.
                        nc.gpsimd.dma_start(od[t], ot)
                    elif variant == "f32r_sw2":
                        # Loads ride both HWDGE rings; stores all SWDGE.
                        nc.gpsimd.dma_start(od[t], ot)
                    else:
                        nc.scalar.dma_start(od[t], ot)
                else:
                    ps = pspool.tile(
                        [128, NT], dt, name=f"ps_{t}", tag="ps"
                    )
                    nc.tensor.matmul(
                        ps[0:1, 0:1],
                        w_sb[0:64, 0:1],
                        w_sb[0:64, 0:1],
                        start=True,
                        stop=True,
                    )
                    for c in (0, 1):
                        lo = 64 * c
                        for j in range(NT // 512):
                            nc.tensor.matmul(
                                ps[lo : lo + 64, 512 * j : 512 * (j + 1)],
                                w_sb[lo : lo + 64, :],
                                xt[lo : lo + 64, 512 * j : 512 * (j + 1)],
                                start=True,
                                stop=True,
                            )
                    ot = iopool.tile([128, NT], dt)
                    if t % 2 == 0:
                        nc.scalar.add(ot, ps, add=b_sb)
                    else:
                        nc.vector.tensor_scalar_add(ot, ps, b_sb)
                    nc.scalar.dma_start(od[t], ot)

    _split_excess_waits(nc)
    return nc


def _get_nc() -> bass.Bass:
    global _NC_CACHE
    if _NC_CACHE is None:
        _NC_CACHE = _build_bass()
    return _NC_CACHE


def _round_f32r(a: np.ndarray) -> np.ndarray:
    """Round fp32 to float32r precision (e8m11: keep top 20 bits, RNE)."""
    u = np.ascontiguousarray(a, dtype=np.float32).view(np.uint32)
    r = (u + np.uint32(0x7FF) + ((u >> np.uint32(12)) & np.uint32(1))) & np.uint32(
        0xFFFFF000
    )
    return r.view(np.float32)


def kernel(x: np.ndarray, weight: np.ndarray, bias: np.ndarray) -> np.ndarray:
    global LAST_RESULTS
    lead_shape = x.shape[:-2]
    rnd = _round_f32r if VARIANT.startswith("f32r") else (
        lambda a: np.ascontiguousarray(a, dtype=np.float32)
    )
    xf = rnd(x).reshape(B_TOTAL, 64)
    NT = _nt_for(VARIANT)
    TILES = B_CORE // (2 * NT)

    m = _build_m(weight)
    wd = rnd(np.concatenate([m, m], axis=0))  # [128, 64]
    bflat = np.asarray(bias, dtype=np.float32).reshape(64)
    bd = np.ascontiguousarray(
        np.concatenate([bflat, bflat]).reshape(128, 1)
    )

    # Pack: partition 64*c+f of tile t on core s holds feature f of batches
    # [s*B_CORE + (2t+c)*NT, ... + NT).
    xp = np.ascontiguousarray(
        xf.reshape(N_CORES, TILES, 2, NT, 64).transpose(0, 1, 2, 4, 3)
    ).reshape(N_CORES, TILES, 128, NT)

    in_maps = [{"xd": xp[s], "wd": wd, "bd": bd} for s in range(N_CORES)]
    nc = _get_nc()
    res = run_bass_kernel_spmd(nc, in_maps, core_ids=list(range(N_CORES)))
    LAST_RESULTS = res

    o = np.stack([res.results[s]["od"] for s in range(N_CORES)])
    out = (
        o.reshape(N_CORES, TILES, 2, 64, NT)
        .transpose(0, 1, 2, 4, 3)
        .reshape(B_TOTAL, 64)
        .reshape(*lead_shape, 8, 8)
    )
    return np.ascontiguousarray(out)



# revision 2
# speedup vs baseline: 2.0187x; 2.0187x over previous
"""CliffordLinear forward on 8 Trainium2 NeuronCores.

The reference computes, for x:[4096,512,8,8], weight:[8,8,8], bias:[8,8]:

    out[b, o, k] = sum_{i,q,p} T[k,p,q] * weight[o,i,p] * x[b, i, q] + bias[o,k]

which is a single GEMM over the flattened feature dims:

    out_flat[b, (o,k)] = x_flat[b, (i,q)] @ M[(i,q), (o,k)] + bias_flat[(o,k)]
    M[(i,q), (o,k)]    = sum_p T[k,p,q] * weight[o,i,p]      (dense 64x64)

Strategy (data-parallel over the batch dim, per the sharding hint):
  - Host: build M (tiny), shard x_flat [2M, 64] into 8 equal row blocks.
  - Host: cast x to fp16 (the kernel is HBM-bound; fp16 halves the traffic
    and its ~5e-4 rel err is far under the 2e-2 gate) and pack each shard
    into [TILES, 128, NT] "feature-major" tiles: tile t holds two column
    blocks of xT = x_flat.T, one on partitions 0:64 and one on 64:128.
  - Device (SPMD, identical NEFF on cores 0-7): stream tiles in via HWDGE
    DMA; the stationary operand is the 128x128 block-diagonal diag(M, M) in
    fp16, so ONE matmul per 512-column chunk computes both halves (fp16
    streams 1 column/cycle through the PE, accumulating in fp32 PSUM).
    PSUM -> SBUF copy fused with the bias add and the fp32->fp16 downcast
    (alternating ScalarE / VectorE), stream fp16 tiles out.
  - Host: upcast to fp32 and unpack to [4096, 512, 8, 8].

The kernel is memory-bound: 32 MB in + 32 MB out per core at ~330-360 GB/s.
"""

import os

import numpy as np

import concourse.bass as bass
import concourse.mybir as mybir
import concourse.tile as tile
from concourse.bass_utils import run_bass_kernel_spmd

N_CORES = 8
B_TOTAL = 4096 * 512
B_CORE = B_TOTAL // N_CORES  # 262144 rows per core

# batch columns per partition half per DMA tile, by build variant
_VARIANT_NT = {"fp16_big": 8192, "fp16_small": 2048}


def _nt_for(variant: str) -> int:
    return _VARIANT_NT.get(variant, 4096)

# Cl(3,0) structure constants: (a x b)_k = sum_{p,q} T[k,p,q] a_p b_q.
_TERMS = [
    (0,0,0, 1),(0,1,1, 1),(0,2,2, 1),(0,3,3, 1),(0,4,4,-1),(0,5,5,-1),(0,6,6,-1),(0,7,7,-1),
    (1,0,1, 1),(1,1,0, 1),(1,2,4,-1),(1,3,5, 1),(1,4,2, 1),(1,5,3,-1),(1,6,7,-1),(1,7,6,-1),
    (2,0,2, 1),(2,1,4, 1),(2,2,0, 1),(2,3,6,-1),(2,4,1,-1),(2,5,7, 1),(2,6,3, 1),(2,7,5,-1),
    (3,0,3, 1),(3,1,5,-1),(3,2,6, 1),(3,3,0, 1),(3,4,7,-1),(3,5,1,-1),(3,6,2,-1),(3,7,4, 1),
    (4,0,4, 1),(4,1,2, 1),(4,2,1,-1),(4,3,7, 1),(4,4,0, 1),(4,5,6,-1),(4,6,5, 1),(4,7,3,-1),
    (5,0,5, 1),(5,1,3,-1),(5,2,7, 1),(5,3,1, 1),(5,4,6, 1),(5,5,0, 1),(5,6,4,-1),(5,7,2,-1),
    (6,0,6, 1),(6,1,7,-1),(6,2,3,-1),(6,3,2, 1),(6,4,5,-1),(6,5,4, 1),(6,6,0, 1),(6,7,1, 1),
    (7,0,7, 1),(7,1,6, 1),(7,2,5,-1),(7,3,4, 1),(7,4,3, 1),(7,5,2,-1),(7,6,1, 1),(7,7,0, 1),
]

# Results of the most recent run_bass_kernel_spmd call (for test harnesses
# that want exec_time_ns / trace paths).
LAST_RESULTS = None

_NC_CACHE = None

VARIANT = os.environ.get("CLIFFORD_KERNEL_VARIANT", "fp16")


def _build_m(weight: np.ndarray) -> np.ndarray:
    t = np.zeros((8, 8, 8), np.float32)
    for k, p, q, s in _TERMS:
        t[k, p, q] = s
    m = np.einsum("kpq,oip->iqok", t, weight.astype(np.float32))
    return np.ascontiguousarray(m.reshape(64, 64), dtype=np.float32)


def _split_excess_waits(nc: bass.Bass, max_waits: int = 1) -> None:
    """Walrus limits the number of sync-wait commands per lowered instruction
    (1 for the PE LDWEIGHTS struct; the tile-context tail Drain with 9+ waits
    also overflows). Move excess waits onto preceding same-engine NOPs, which
    execute their waits in program order before the instruction."""
    pe_ops = ("Matmult", "Ldweights")
    n = 0
    for f in nc.m.functions:
        for blk in f.blocks:
            il = blk.instructions
            idx = 0
            while idx < len(il):
                inst = il[idx]
                si = inst.sync_info
                if si is None or not si.on_wait:
                    idx += 1
                    continue
                limit = 1 if inst.opcode in pe_ops else max_waits
                waits = list(si.on_wait)
                if len(waits) <= limit:
                    idx += 1
                    continue
                keep = waits[-limit:]
                extra = waits[:-limit]
                for j in range(0, len(extra), max_waits):
                    n += 1
                    nop = mybir.InstNoOp(
                        name=f"I-waitsplit-{n}",
                        sync_info=mybir.SyncInfo(
                            on_wait=extra[j : j + max_waits], on_update=[]
                        ),
                        bass_nofuse=True,
                        engine=inst.engine,
                    )
                    il.insert(idx, nop)
                    idx += 1
                inst.sync_info = mybir.SyncInfo(on_wait=keep, on_update=si.on_update)
                idx += 1


def _build_bass(variant: str | None = None) -> bass.Bass:
    variant = variant or VARIANT
    NT = _nt_for(variant)
    TILES = B_CORE // (2 * NT)
    nc = bass.Bass()
    f16 = mybir.dt.float16
    f32 = mybir.dt.float32
    xd = nc.dram_tensor("xd", [TILES, 128, NT], f16, kind="ExternalInput")
    wd = nc.dram_tensor("wd", [128, 128], f16, kind="ExternalInput")
    bd = nc.dram_tensor("bd", [128, 1], f32, kind="ExternalInput")
    od = nc.dram_tensor("od", [TILES, 128, NT], f16, kind="ExternalOutput")

    with tile.TileContext(nc) as tc:
        with (
            tc.tile_pool(name="cpool", bufs=1) as cpool,
            tc.tile_pool(name="iopool", bufs=6) as iopool,
            tc.tile_pool(name="pspool", bufs=4, space="PSUM") as pspool,
        ):
            # Stationary operand: block-diag(M, M), so one matmul covers both
            # partition halves of the packed input tile.
            w_sb = cpool.tile([128, 128], f16)
            nc.sync.dma_start(w_sb, wd[:])
            b_sb = cpool.tile([128, 1], f32)
            nc.sync.dma_start(b_sb, bd[:])

            # Prologue touches fold the bias DMA wait into each copy
            # engine's clock once, instead of onto a steady-state op.
            # (_split_excess_waits legalizes any remaining multi-wait
            # instruction by spilling waits onto same-engine NOPs.)
            scr_a = cpool.tile([128, 1], f32)
            nc.scalar.copy(scr_a, b_sb)
            scr_v = cpool.tile([128, 1], f32)
            nc.vector.tensor_copy(scr_v, b_sb)

            NS = 512  # matmul moving-operand chunk; [128, 512] f32 = 1 PSUM bank
            for t in range(TILES):
                xt = iopool.tile([128, NT], f16)
                nc.sync.dma_start(xt, xd[t])
                ot = iopool.tile([128, NT], f16)
                for s in range(NT // NS):
                    ps = pspool.tile([128, NS], f32, name=f"ps_{t}_{s}", tag="ps")
                    nc.tensor.matmul(
                        ps,
                        w_sb,
                        xt[:, s * NS : (s + 1) * NS],
                        start=True,
                        stop=True,
                    )
                    # PSUM -> SBUF copy + bias + fp32->fp16 downcast,
                    # alternating engines so neither becomes the bottleneck.
                    if s % 2 == 0:
                        nc.scalar.add(
                            ot[:, s * NS : (s + 1) * NS], ps, add=b_sb
                        )
                    else:
                        nc.vector.tensor_scalar_add(
                            ot[:, s * NS : (s + 1) * NS], ps, b_sb
                        )
                if variant == "fp16_sw":
                    nc.gpsimd.dma_start(od[t], ot)
                else:
                    nc.scalar.dma_start(od[t], ot)

    _split_excess_waits(nc)
    return nc


def _get_nc() -> bass.Bass:
    global _NC_CACHE
    if _NC_CACHE is None:
        _NC_CACHE = _build_bass()
    return _NC_CACHE


def kernel(x: np.ndarray, weight: np.ndarray, bias: np.ndarray) -> np.ndarray:
    global LAST_RESULTS
    lead_shape = x.shape[:-2]
    NT = _nt_for(VARIANT)
    TILES = B_CORE // (2 * NT)

    xf = np.asarray(x, dtype=np.float32).reshape(B_TOTAL, 64).astype(np.float16)

    m = _build_m(weight)
    wbig = np.zeros((128, 128), np.float32)
    wbig[0:64, 0:64] = m
    wbig[64:128, 64:128] = m
    wd = wbig.astype(np.float16)
    bflat = np.asarray(bias, dtype=np.float32).reshape(64)
    bd = np.ascontiguousarray(
        np.concatenate([bflat, bflat]).reshape(128, 1)
    )

    # Pack: partition 64*c+f of tile t on core s holds feature f of batches
    # [s*B_CORE + (2t+c)*NT, ... + NT).
    xp = np.ascontiguousarray(
        xf.reshape(N_CORES, TILES, 2, NT, 64).transpose(0, 1, 2, 4, 3)
    ).reshape(N_CORES, TILES, 128, NT)

    in_maps = [{"xd": xp[s], "wd": wd, "bd": bd} for s in range(N_CORES)]
    nc = _get_nc()
    res = run_bass_kernel_spmd(nc, in_maps, core_ids=list(range(N_CORES)))
    LAST_RESULTS = res

    o = np.stack([res.results[s]["od"] for s in range(N_CORES)])
    out = (
        o.reshape(N_CORES, TILES, 2, 64, NT)
        .transpose(0, 1, 2, 4, 3)
        .reshape(B_TOTAL, 64)
        .astype(np.float32)
        .reshape(*lead_shape, 8, 8)
    )
    return np.ascontiguousarray(out)


# revision 6
# speedup vs baseline: 2.1559x; 1.0680x over previous
"""CliffordLinear forward on 8 Trainium2 NeuronCores.

The reference computes, for x:[4096,512,8,8], weight:[8,8,8], bias:[8,8]:

    out[b, o, k] = sum_{i,q,p} T[k,p,q] * weight[o,i,p] * x[b, i, q] + bias[o,k]

which is a single GEMM over the flattened feature dims:

    out_flat[b, (o,k)] = x_flat[b, (i,q)] @ M[(i,q), (o,k)] + bias_flat[(o,k)]
    M[(i,q), (o,k)]    = sum_p T[k,p,q] * weight[o,i,p]      (dense 64x64)

Strategy (data-parallel over the batch dim, per the sharding hint):
  - Host: build M (tiny), shard x_flat [2M, 64] into 8 equal row blocks.
  - Host: cast x to fp16 (the kernel is HBM-bound; fp16 halves the traffic
    and its ~5e-4 rel err is far under the 2e-2 gate) and pack each shard
    into [TILES, 128, NT] "feature-major" tiles: tile t holds two column
    blocks of xT = x_flat.T, one on partitions 0:64 and one on 64:128.
  - Device (SPMD, identical NEFF on cores 0-7): stream tiles in via HWDGE
    DMA; the stationary operand is the 128x128 block-diagonal diag(M, M) in
    fp16, so ONE matmul per 512-column chunk computes both halves (fp16
    streams 1 column/cycle through the PE, accumulating in fp32 PSUM).
    PSUM -> SBUF copy fused with the bias add and the fp32->fp16 downcast
    (alternating ScalarE / VectorE), stream fp16 tiles out.
  - Host: upcast to fp32 and unpack to [4096, 512, 8, 8].

The kernel is memory-bound: 32 MB in + 32 MB out per core at ~330-360 GB/s.
"""

import os

import numpy as np

import concourse.bass as bass
import concourse.mybir as mybir
import concourse.tile as tile
from concourse.bass_utils import run_bass_kernel_spmd

N_CORES = 8
B_TOTAL = 4096 * 512
B_CORE = B_TOTAL // N_CORES  # 262144 rows per core
HALF = B_CORE // 2  # 131072 batch columns per partition half

# Variable tile schedule (columns per DMA): small head tiles prime the
# store pipeline early, small tail tiles shrink the drain, big middle
# tiles amortize per-DMA issue cost (~0.6us) and use 16KB descriptors.
_SCHEDULES = {
    "fp16": [2048, 2048, 4096] + [8192] * 14 + [4096, 2048, 2048],
    "fp16_flat": [4096] * 32,
}


def _schedule_for(variant: str) -> list[int]:
    sched = _SCHEDULES.get(variant, _SCHEDULES["fp16"])
    assert sum(sched) == HALF, (sum(sched), HALF)
    return sched

# Cl(3,0) structure constants: (a x b)_k = sum_{p,q} T[k,p,q] a_p b_q.
_TERMS = [
    (0,0,0, 1),(0,1,1, 1),(0,2,2, 1),(0,3,3, 1),(0,4,4,-1),(0,5,5,-1),(0,6,6,-1),(0,7,7,-1),
    (1,0,1, 1),(1,1,0, 1),(1,2,4,-1),(1,3,5, 1),(1,4,2, 1),(1,5,3,-1),(1,6,7,-1),(1,7,6,-1),
    (2,0,2, 1),(2,1,4, 1),(2,2,0, 1),(2,3,6,-1),(2,4,1,-1),(2,5,7, 1),(2,6,3, 1),(2,7,5,-1),
    (3,0,3, 1),(3,1,5,-1),(3,2,6, 1),(3,3,0, 1),(3,4,7,-1),(3,5,1,-1),(3,6,2,-1),(3,7,4, 1),
    (4,0,4, 1),(4,1,2, 1),(4,2,1,-1),(4,3,7, 1),(4,4,0, 1),(4,5,6,-1),(4,6,5, 1),(4,7,3,-1),
    (5,0,5, 1),(5,1,3,-1),(5,2,7, 1),(5,3,1, 1),(5,4,6, 1),(5,5,0, 1),(5,6,4,-1),(5,7,2,-1),
    (6,0,6, 1),(6,1,7,-1),(6,2,3,-1),(6,3,2, 1),(6,4,5,-1),(6,5,4, 1),(6,6,0, 1),(6,7,1, 1),
    (7,0,7, 1),(7,1,6, 1),(7,2,5,-1),(7,3,4, 1),(7,4,3, 1),(7,5,2,-1),(7,6,1, 1),(7,7,0, 1),
]

# Results of the most recent run_bass_kernel_spmd call (for test harnesses
# that want exec_time_ns / trace paths).
LAST_RESULTS = None

_NC_CACHE = None

VARIANT = os.environ.get("CLIFFORD_KERNEL_VARIANT", "fp16")


def _build_m(weight: np.ndarray) -> np.ndarray:
    t = np.zeros((8, 8, 8), np.float32)
    for k, p, q, s in _TERMS:
        t[k, p, q] = s
    m = np.einsum("kpq,oip->iqok", t, weight.astype(np.float32))
    return np.ascontiguousarray(m.reshape(64, 64), dtype=np.float32)


def _split_excess_waits(nc: bass.Bass, max_waits: int = 1) -> None:
    """Walrus limits the number of sync-wait commands per lowered instruction
    (1 for the PE LDWEIGHTS struct; the tile-context tail Drain with 9+ waits
    also overflows). Move excess waits onto preceding same-engine NOPs, which
    execute their waits in program order before the instruction."""
    pe_ops = ("Matmult", "Ldweights")
    n = 0
    for f in nc.m.functions:
        for blk in f.blocks:
            il = blk.instructions
            idx = 0
            while idx < len(il):
                inst = il[idx]
                si = inst.sync_info
                if si is None or not si.on_wait:
                    idx += 1
                    continue
                limit = 1 if inst.opcode in pe_ops else max_waits
                waits = list(si.on_wait)
                if len(waits) <= limit:
                    idx += 1
                    continue
                keep = waits[-limit:]
                extra = waits[:-limit]
                for j in range(0, len(extra), max_waits):
                    n += 1
                    nop = mybir.InstNoOp(
                        name=f"I-waitsplit-{n}",
                        sync_info=mybir.SyncInfo(
                            on_wait=extra[j : j + max_waits], on_update=[]
                        ),
                        bass_nofuse=True,
                        engine=inst.engine,
                    )
                    il.insert(idx, nop)
                    idx += 1
                inst.sync_info = mybir.SyncInfo(on_wait=keep, on_update=si.on_update)
                idx += 1


def _build_bass(variant: str | None = None) -> bass.Bass:
    variant = variant or VARIANT
    sched = _schedule_for(variant)
    NTMAX = max(sched)
    nc = bass.Bass()
    f16 = mybir.dt.float16
    f32 = mybir.dt.float32
    xd = nc.dram_tensor("xd", [128, HALF], f16, kind="ExternalInput")
    wd = nc.dram_tensor("wd", [128, 128], f16, kind="ExternalInput")
    bd = nc.dram_tensor("bd", [128, 1], f32, kind="ExternalInput")
    od = nc.dram_tensor("od", [128, HALF], f16, kind="ExternalOutput")

    with tile.TileContext(nc) as tc:
        with (
            tc.tile_pool(name="cpool", bufs=1) as cpool,
            tc.tile_pool(name="iopool", bufs=5) as iopool,
            tc.tile_pool(name="pspool", bufs=4, space="PSUM") as pspool,
        ):
            # Stationary operand: block-diag(M, M), so one matmul covers both
            # partition halves of the packed input tile. Loaded on the scalar
            # HWDGE ring so the sync ring's first DMA is tile 0's load.
            w_sb = cpool.tile([128, 128], f16)
            nc.scalar.dma_start(w_sb, wd[:])
            b_sb = cpool.tile([128, 1], f32)
            nc.scalar.dma_start(b_sb, bd[:])

            # Prologue touches fold the bias DMA wait into each copy
            # engine's clock once, instead of onto a steady-state op.
            # (_split_excess_waits legalizes any remaining multi-wait
            # instruction by spilling waits onto same-engine NOPs.)
            scr_a = cpool.tile([128, 1], f32)
            nc.scalar.copy(scr_a, b_sb)
            scr_v = cpool.tile([128, 1], f32)
            nc.vector.tensor_copy(scr_v, b_sb)

            NS = 512  # matmul moving-operand chunk; [128, 512] f32 = 1 PSUM bank
            c0 = 0
            for t, nt in enumerate(sched):
                xt = iopool.tile([128, NTMAX], f16)
                nc.sync.dma_start(xt[:, 0:nt], xd[:, c0 : c0 + nt])
                ot = iopool.tile([128, NTMAX], f16)
                for s in range(nt // NS):
                    ps = pspool.tile([128, NS], f32, name=f"ps_{t}_{s}", tag="ps")
                    nc.tensor.matmul(
                        ps,
                        w_sb,
                        xt[:, s * NS : (s + 1) * NS],
                        start=True,
                        stop=True,
                    )
                    # PSUM -> SBUF copy + bias + fp32->fp16 downcast,
                    # alternating engines so neither becomes the bottleneck.
                    if s % 2 == 0:
                        nc.scalar.add(
                            ot[:, s * NS : (s + 1) * NS], ps, add=b_sb
                        )
                    else:
                        nc.vector.tensor_scalar_add(
                            ot[:, s * NS : (s + 1) * NS], ps, b_sb
                        )
                nc.scalar.dma_start(od[:, c0 : c0 + nt], ot[:, 0:nt])
                c0 += nt

    _split_excess_waits(nc)
    return nc


def _get_nc() -> bass.Bass:
    global _NC_CACHE
    if _NC_CACHE is None:
        _NC_CACHE = _build_bass()
    return _NC_CACHE


def kernel(x: np.ndarray, weight: np.ndarray, bias: np.ndarray) -> np.ndarray:
    global LAST_RESULTS
    lead_shape = x.shape[:-2]

    xf = np.asarray(x, dtype=np.float32).reshape(B_TOTAL, 64).astype(np.float16)

    m = _build_m(weight)
    wbig = np.zeros((128, 128), np.float32)
    wbig[0:64, 0:64] = m
    wbig[64:128, 64:128] = m
    wd = wbig.astype(np.float16)
    bflat = np.asarray(bias, dtype=np.float32).reshape(64)
    bd = np.ascontiguousarray(
        np.concatenate([bflat, bflat]).reshape(128, 1)
    )

    # Pack: partition 64*c+f, column j on core s holds feature f of batch
    # s*B_CORE + c*HALF + j (feature-major, contraction dim on partitions).
    xp = np.ascontiguousarray(
        xf.reshape(N_CORES, 2, HALF, 64).transpose(0, 1, 3, 2)
    ).reshape(N_CORES, 128, HALF)

    in_maps = [{"xd": xp[s], "wd": wd, "bd": bd} for s in range(N_CORES)]
    nc = _get_nc()
    res = run_bass_kernel_spmd(nc, in_maps, core_ids=list(range(N_CORES)))
    LAST_RESULTS = res

    o = np.stack([res.results[s]["od"] for s in range(N_CORES)])
    out = (
        o.reshape(N_CORES, 2, 64, HALF)
        .transpose(0, 1, 3, 2)
        .reshape(B_TOTAL, 64)
        .astype(np.float32)
        .reshape(*lead_shape, 8, 8)
    )
    return np.ascontiguousarray(out)


# revision 8
# speedup vs baseline: 2.4162x; 1.1207x over previous
"""CliffordLinear forward on 8 Trainium2 NeuronCores.

The reference computes, for x:[4096,512,8,8], weight:[8,8,8], bias:[8,8]:

    out[b, o, k] = sum_{i,q,p} T[k,p,q] * weight[o,i,p] * x[b, i, q] + bias[o,k]

which is a single GEMM over the flattened feature dims:

    out_flat[b, (o,k)] = x_flat[b, (i,q)] @ M[(i,q), (o,k)] + bias_flat[(o,k)]
    M[(i,q), (o,k)]    = sum_p T[k,p,q] * weight[o,i,p]      (dense 64x64)

Strategy (data-parallel over the batch dim, per the sharding hint):
  - Host: build M (tiny), shard x_flat [2M, 64] into 8 equal row blocks.
  - Host: cast x to fp16 (the kernel is HBM-bound; fp16 halves the traffic
    and its ~5e-4 rel err is far under the 2e-2 gate) and pack each shard
    into [TILES, 128, NT] "feature-major" tiles: tile t holds two column
    blocks of xT = x_flat.T, one on partitions 0:64 and one on 64:128.
  - Device (SPMD, identical NEFF on cores 0-7): stream tiles in via HWDGE
    DMA; the stationary operand is the 128x128 block-diagonal diag(M, M) in
    fp16, so ONE matmul per 512-column chunk computes both halves (fp16
    streams 1 column/cycle through the PE, accumulating in fp32 PSUM).
    PSUM -> SBUF copy fused with the bias add and the fp32->fp16 downcast
    (alternating ScalarE / VectorE), stream fp16 tiles out.
  - Host: upcast to fp32 and unpack to [4096, 512, 8, 8].

The kernel is memory-bound: 32 MB in + 32 MB out per core at ~330-360 GB/s.
"""

import os

import numpy as np

import concourse.bass as bass
import concourse.mybir as mybir
import concourse.tile as tile
from concourse.bass_utils import run_bass_kernel_spmd

N_CORES = 8
B_TOTAL = 4096 * 512
B_CORE = B_TOTAL // N_CORES  # 262144 rows per core
HALF = B_CORE // 2  # 131072 batch columns per partition half

# Variable tile schedule (columns per DMA): small head tiles prime the
# store pipeline early, small tail tiles shrink the drain, big middle
# tiles amortize per-DMA issue cost (~0.6us) and use 16KB descriptors.
_SCHEDULES = {
    "fp16": [2048, 2048, 4096] + [8192] * 14 + [4096, 2048, 2048],
    "fp16_flat": [4096] * 32,
}


def _schedule_for(variant: str) -> list[int]:
    sched = _SCHEDULES.get(variant, _SCHEDULES["fp16"])
    assert sum(sched) == HALF, (sum(sched), HALF)
    return sched

# Cl(3,0) structure constants: (a x b)_k = sum_{p,q} T[k,p,q] a_p b_q.
_TERMS = [
    (0,0,0, 1),(0,1,1, 1),(0,2,2, 1),(0,3,3, 1),(0,4,4,-1),(0,5,5,-1),(0,6,6,-1),(0,7,7,-1),
    (1,0,1, 1),(1,1,0, 1),(1,2,4,-1),(1,3,5, 1),(1,4,2, 1),(1,5,3,-1),(1,6,7,-1),(1,7,6,-1),
    (2,0,2, 1),(2,1,4, 1),(2,2,0, 1),(2,3,6,-1),(2,4,1,-1),(2,5,7, 1),(2,6,3, 1),(2,7,5,-1),
    (3,0,3, 1),(3,1,5,-1),(3,2,6, 1),(3,3,0, 1),(3,4,7,-1),(3,5,1,-1),(3,6,2,-1),(3,7,4, 1),
    (4,0,4, 1),(4,1,2, 1),(4,2,1,-1),(4,3,7, 1),(4,4,0, 1),(4,5,6,-1),(4,6,5, 1),(4,7,3,-1),
    (5,0,5, 1),(5,1,3,-1),(5,2,7, 1),(5,3,1, 1),(5,4,6, 1),(5,5,0, 1),(5,6,4,-1),(5,7,2,-1),
    (6,0,6, 1),(6,1,7,-1),(6,2,3,-1),(6,3,2, 1),(6,4,5,-1),(6,5,4, 1),(6,6,0, 1),(6,7,1, 1),
    (7,0,7, 1),(7,1,6, 1),(7,2,5,-1),(7,3,4, 1),(7,4,3, 1),(7,5,2,-1),(7,6,1, 1),(7,7,0, 1),
]

# Results of the most recent run_bass_kernel_spmd call (for test harnesses
# that want exec_time_ns / trace paths).
LAST_RESULTS = None

_NC_CACHE = None

VARIANT = os.environ.get("CLIFFORD_KERNEL_VARIANT", "fp16")


def _build_m(weight: np.ndarray) -> np.ndarray:
    t = np.zeros((8, 8, 8), np.float32)
    for k, p, q, s in _TERMS:
        t[k, p, q] = s
    m = np.einsum("kpq,oip->iqok", t, weight.astype(np.float32))
    return np.ascontiguousarray(m.reshape(64, 64), dtype=np.float32)


def _split_excess_waits(nc: bass.Bass, max_waits: int = 1) -> None:
    """Walrus limits the number of sync-wait commands per lowered instruction
    (1 for the PE LDWEIGHTS struct; the tile-context tail Drain with 9+ waits
    also overflows). Move excess waits onto preceding same-engine NOPs, which
    execute their waits in program order before the instruction."""
    pe_ops = ("Matmult", "Ldweights")
    n = 0
    for f in nc.m.functions:
        for blk in f.blocks:
            il = blk.instructions
            idx = 0
            while idx < len(il):
                inst = il[idx]
                si = inst.sync_info
                if si is None or not si.on_wait:
                    idx += 1
                    continue
                limit = 1 if inst.opcode in pe_ops else max_waits
                waits = list(si.on_wait)
                if len(waits) <= limit:
                    idx += 1
                    continue
                keep = waits[-limit:]
                extra = waits[:-limit]
                for j in range(0, len(extra), max_waits):
                    n += 1
                    nop = mybir.InstNoOp(
                        name=f"I-waitsplit-{n}",
                        sync_info=mybir.SyncInfo(
                            on_wait=extra[j : j + max_waits], on_update=[]
                        ),
                        bass_nofuse=True,
                        engine=inst.engine,
                    )
                    il.insert(idx, nop)
                    idx += 1
                inst.sync_info = mybir.SyncInfo(on_wait=keep, on_update=si.on_update)
                idx += 1


def _build_bass(variant: str | None = None) -> bass.Bass:
    variant = variant or VARIANT
    sched = _schedule_for(variant)
    NTMAX = max(sched)
    nc = bass.Bass()
    f16 = mybir.dt.float16
    f32 = mybir.dt.float32
    xd = nc.dram_tensor("xd", [128, HALF], f16, kind="ExternalInput")
    wd = nc.dram_tensor("wd", [128, 128], f16, kind="ExternalInput")
    bd = nc.dram_tensor("bd", [128, 1], f32, kind="ExternalInput")
    od = nc.dram_tensor("od", [128, HALF], f16, kind="ExternalOutput")

    with tile.TileContext(nc) as tc:
        with (
            tc.tile_pool(name="cpool", bufs=1) as cpool,
            tc.tile_pool(name="iopool", bufs=5) as iopool,
            tc.tile_pool(name="pspool", bufs=6, space="PSUM") as pspool,
        ):
            # Stationary operand: block-diag(M, M), so one matmul covers both
            # partition halves of the packed input tile. Loaded on the scalar
            # HWDGE ring so the sync ring's first DMA is tile 0's load.
            w_sb = cpool.tile([128, 128], f16)
            nc.scalar.dma_start(w_sb, wd[:])
            b_sb = cpool.tile([128, 1], f32)
            nc.scalar.dma_start(b_sb, bd[:])

            # Prologue touches fold the bias DMA wait into each copy
            # engine's clock once, instead of onto a steady-state op.
            # (_split_excess_waits legalizes any remaining multi-wait
            # instruction by spilling waits onto same-engine NOPs.)
            scr_a = cpool.tile([128, 1], f32)
            nc.scalar.copy(scr_a, b_sb)
            scr_v = cpool.tile([128, 1], f32)
            nc.vector.tensor_copy(scr_v, b_sb)

            NS = 512  # matmul moving-operand chunk; [128, 512] f32 = 1 PSUM bank
            c0 = 0
            for t, nt in enumerate(sched):
                xt = iopool.tile([128, NTMAX], f16)
                nc.sync.dma_start(xt[:, 0:nt], xd[:, c0 : c0 + nt])
                ot = iopool.tile([128, NTMAX], f16)
                for s in range(nt // NS):
                    ps = pspool.tile([128, NS], f32, name=f"ps_{t}_{s}", tag="ps")
                    nc.tensor.matmul(
                        ps,
                        w_sb,
                        xt[:, s * NS : (s + 1) * NS],
                        start=True,
                        stop=True,
                    )
                    # PSUM -> SBUF copy + bias + fp32->fp16 downcast,
                    # alternating engines so neither becomes the bottleneck.
                    if s % 2 == 0:
                        nc.scalar.add(
                            ot[:, s * NS : (s + 1) * NS], ps, add=b_sb
                        )
                    else:
                        nc.vector.tensor_scalar_add(
                            ot[:, s * NS : (s + 1) * NS], ps, b_sb
                        )
                # Alternate stores between the scalar HWDGE ring and the
                # gpsimd SWDGE ring: two store queues double the store
                # stream's share of the 16 shared SDMA engines (engines
                # round-robin across queues at packet granularity), which
                # matters because the store stream is the critical path.
                if t % 2 == 0:
                    nc.scalar.dma_start(od[:, c0 : c0 + nt], ot[:, 0:nt])
                else:
                    nc.gpsimd.dma_start(od[:, c0 : c0 + nt], ot[:, 0:nt])
                c0 += nt

    _split_excess_waits(nc)
    return nc


def _get_nc() -> bass.Bass:
    global _NC_CACHE
    if _NC_CACHE is None:
        _NC_CACHE = _build_bass()
    return _NC_CACHE


def kernel(x: np.ndarray, weight: np.ndarray, bias: np.ndarray) -> np.ndarray:
    global LAST_RESULTS
    lead_shape = x.shape[:-2]

    xf = np.asarray(x, dtype=np.float32).reshape(B_TOTAL, 64).astype(np.float16)

    m = _build_m(weight)
    wbig = np.zeros((128, 128), np.float32)
    wbig[0:64, 0:64] = m
    wbig[64:128, 64:128] = m
    wd = wbig.astype(np.float16)
    bflat = np.asarray(bias, dtype=np.float32).reshape(64)
    bd = np.ascontiguousarray(
        np.concatenate([bflat, bflat]).reshape(128, 1)
    )

    # Pack: partition 64*c+f, column j on core s holds feature f of batch
    # s*B_CORE + c*HALF + j (feature-major, contraction dim on partitions).
    xp = np.ascontiguousarray(
        xf.reshape(N_CORES, 2, HALF, 64).transpose(0, 1, 3, 2)
    ).reshape(N_CORES, 128, HALF)

    in_maps = [{"xd": xp[s], "wd": wd, "bd": bd} for s in range(N_CORES)]
    nc = _get_nc()
    res = run_bass_kernel_spmd(nc, in_maps, core_ids=list(range(N_CORES)))
    LAST_RESULTS = res

    o = np.stack([res.results[s]["od"] for s in range(N_CORES)])
    out = (
        o.reshape(N_CORES, 2, 64, HALF)
        .transpose(0, 1, 3, 2)
        .reshape(B_TOTAL, 64)
        .astype(np.float32)
        .reshape(*lead_shape, 8, 8)
    )
    return np.ascontiguousarray(out)
